# revision 2
# baseline (speedup 1.0000x reference)
"""Trainium2 Bass kernel for nn_CriticGraphPolicy (single-limb critic GNN).

Live graph per row (state s[32], action a[1]):
  u  = [s;a;1] @ Wup1;  xu = u/||u||;  h1 = tanh(xu)
  h2 = tanh([h1;1] @ Wup2);  v = [h2;1] @ Wup3;  m = v/||v||
  x37 = [m; s[:3]; a; 1]
  z1 = relu(x37 @ Wq1);  z2 = relu([z1;1] @ Wq2);  xq = z2@w3 + b3
for q in {q1, q2}.

Batch layout: row r of a core's 8192 = g*2048 + p*16 + j  (g group,
p partition, j = qq*4 + n; chunk c = g*4 + qq covers 512 rows as
columns n*128 + p).  Host un-permutes at the end and adds b3.

Structure: 3 passes so the Scalar engine never ping-pongs activation
tables (sqrt is batched between passes; tanh/square/relu/copy share
tables within each pass):
  pass A (per group g): load+transpose input, up1, |u|^2 row-sums
  sqrt+recip (all 64 rowgroups at once)
  pass B: xu=u*rinv, tanh, up2, tanh, up3, |v|^2 sums
  sqrt+recip
  pass C (per 512-chunk, software-pipelined one chunk deep so the PE
  never waits on relu results): msg=v*rinv2, build x37, L1(fm), relu,
  then L2(bm) of the PREVIOUS chunk + fused q3 (relu*w3+rowsum in one
  scalar_tensor_tensor on DVE; two of eight via Scalar relu + GpSimd
  mul/reduce to balance engines).
"""

import numpy as np
import ml_dtypes

import concourse.bass as bass
import concourse.bacc as bacc
import concourse.mybir as mybir
from concourse import tile
from concourse.bass_utils import run_bass_kernel_spmd

F32 = mybir.dt.float32
BF16 = mybir.dt.bfloat16

NCORES = 8
B = 65536
BC = B // NCORES          # 8192 rows per core
NG = 4                    # groups of 2048 rows (super-chunks for A/B)
GR = BC // NG             # 2048
NCH = 16                  # pass-C chunks of 512

_prog_cache = {}


def build_program():
    if "nc" in _prog_cache:
        return _prog_cache["nc"]
    nc = bacc.Bacc("TRN2", target_bir_lowering=False, debug=False,
                   num_devices=NCORES)

    state = nc.dram_tensor("state", [BC, 32], F32, kind="ExternalInput").ap()
    actp = nc.dram_tensor("actp", [128, 128], BF16, kind="ExternalInput").ap()
    wup1d = nc.dram_tensor("wup1", [34, 64], BF16, kind="ExternalInput").ap()
    wup2d = nc.dram_tensor("wup2", [65, 64], BF16, kind="ExternalInput").ap()
    wup3d = nc.dram_tensor("wup3", [65, 32], BF16, kind="ExternalInput").ap()
    # q-head weights: L1 main tiles [37, 384] per q + merged extra [37, 48]
    wq1d = [nc.dram_tensor(f"w{q}1", [37, 384], BF16, kind="ExternalInput").ap()
            for q in ("a", "b")]
    wq1xd = nc.dram_tensor("wq1x", [37, 48], BF16, kind="ExternalInput").ap()
    # L2: [417, 300] per q (400 z1 + 16 pad + bias row at 416)
    wq2d = [nc.dram_tensor(f"w{q}2", [417, 300], BF16, kind="ExternalInput").ap()
            for q in ("a", "b")]
    w3d = [nc.dram_tensor(f"w{q}3", [128, 300], BF16, kind="ExternalInput").ap()
           for q in ("a", "b")]
    idbf = nc.dram_tensor("idbf", [128, 128], BF16, kind="ExternalInput").ap()
    out = nc.dram_tensor("out", [128, 128], F32, kind="ExternalOutput").ap()

    AF = mybir.ActivationFunctionType
    ALU = mybir.AluOpType
    KS = [128, 128, 128, 33]          # L2 k-tiles (400 z1 + 16 pad + bias@32)

    with tile.TileContext(nc) as tc:
        with (
            tc.tile_pool(name="wp", bufs=1) as wp,
            tc.tile_pool(name="rot", bufs=2) as rp,
        ):
            # ---- static weights / persistent state ----
            def ldw(tag, src, shape):
                t = wp.tile(shape, BF16, tag=tag, name=tag)
                nc.sync.dma_start(t[:], src)
                return t
            w_up1 = ldw("wup1", wup1d[:], [34, 64])
            w_up2 = ldw("wup2", wup2d[:], [65, 64])
            w_up3 = ldw("wup3", wup3d[:], [65, 32])
            w_q1 = [ldw(f"wq1_{q}", wq1d[q][:], [37, 384]) for q in range(2)]
            w_q1x = ldw("wq1x", wq1xd[:], [37, 48])
            w_q2 = []
            for q in range(2):
                tiles = []
                off = 0
                for k in range(4):
                    t = ldw(f"wq2_{q}_{k}", wq2d[q][off:off + KS[k], :],
                            [KS[k], 300])
                    tiles.append(t)
                    off += KS[k]
                w_q2.append(tiles)
            w_3 = [ldw(f"w3_{q}", w3d[q][:], [128, 300]) for q in range(2)]
            id_bf = ldw("idbf", idbf[:], [128, 128])
            act_sb = ldw("actp", actp[:], [128, 128])

            gsb = []
            for g in range(NG):
                t = wp.tile([128, GR * 32 // 128], F32, tag=f"gsb{g}",
                            name=f"gsb{g}")
                src = state[g * GR:(g + 1) * GR, :].rearrange(
                    "(p j) f -> p (j f)", p=128)
                nc.sync.dma_start(t[:], src)
                gsb.append(t)

            u_sb = [wp.tile([128, 1024], BF16, tag=f"usb{g}", name=f"usb{g}")
                    for g in range(NG)]
            v_sb = [wp.tile([128, 512], BF16, tag=f"vsb{g}", name=f"vsb{g}")
                    for g in range(NG)]
            s1a = wp.tile([128, 64], F32, tag="s1a", name="s1a")
            s2a = wp.tile([128, 64], F32, tag="s2a", name="s2a")
            nrm = wp.tile([128, 64], F32, tag="nrm", name="nrm")
            rinv1 = wp.tile([128, 64], F32, tag="rinv1", name="rinv1")
            rinv2 = wp.tile([128, 64], F32, tag="rinv2", name="rinv2")
            xq = wp.tile([128, 128], F32, tag="xq", name="xq")

            # h1f/h2f feature-major activations (+ones row), parity-buffered
            hf = {}
            for nm in ("h1f", "h2f"):
                for par in range(2):
                    t = wp.tile([65, 2048], BF16, tag=f"{nm}_{par}",
                                name=f"{nm}_{par}")
                    nc.vector.memset(t[64:65, :], 1.0)
                    hf[(nm, par)] = t
            # z1 tiles (feature-major relu outputs), parity-buffered
            z1sb = {}
            for q in range(2):
                for par in range(2):
                    for k in range(4):
                        t = wp.tile([KS[k], 512], BF16, tag=f"z1_{q}{k}{par}",
                                    name=f"z1_{q}{k}{par}")
                        if k == 3:
                            nc.vector.memset(t[:], 0.0)
                            nc.vector.memset(t[32:33, :], 1.0)  # bias row
                        z1sb[(q, k, par)] = t

            # ================= PASS A =================
            with tc.tile_pool(name="pA", bufs=2, space="PSUM") as pa:
                for g in range(NG):
                    xl = rp.tile([128, 16 * 34], BF16, tag="xl", name="xl")
                    nc.gpsimd.tensor_copy(
                        xl[:].rearrange("p (j f) -> p j f", j=16)[:, :, 0:32],
                        gsb[g][:].rearrange("p (j f) -> p j f", j=16))
                    nc.gpsimd.tensor_copy(
                        xl[:].rearrange("p (j f) -> p j f", j=16)[:, :, 32:34],
                        act_sb[:].rearrange("p (c j t) -> p c j t", c=16, j=4)
                        [:, g * 4:(g + 1) * 4, :, :].rearrange(
                            "p c j t -> p (c j) t"))
                    x34t = [pa.tile([34, 1024], BF16, tag="x34t", name="x34t")
                            for _ in range(2)]
                    for j in range(16):
                        nc.tensor.matmul(
                            x34t[j // 8][:, (j % 8) * 128:(j % 8 + 1) * 128],
                            xl[:, j * 34:(j + 1) * 34], id_bf[:],
                            is_transpose=True, skip_group_check=True)
                    x34 = rp.tile([34, 2048], BF16, tag="x34", name="x34")
                    for h in range(2):
                        nc.vector.tensor_copy(
                            x34[:, h * 1024:(h + 1) * 1024], x34t[h][:])
                    u_ps = [pa.tile([128, 512], F32, tag="u", name="u")
                            for _ in range(2)]
                    for j in range(16):
                        nc.tensor.matmul(
                            u_ps[j // 8][:, (j % 8) * 64:(j % 8 + 1) * 64],
                            x34[:, j * 128:(j + 1) * 128], w_up1[:],
                            skip_group_check=True)
                    for h in range(2):
                        usq = rp.tile([128, 512], BF16, tag="usq", name="usq")
                        nc.scalar.activation(usq[:], u_ps[h][:], AF.Square)
                        nc.vector.tensor_reduce(
                            s1a[:, g * 16 + h * 8:g * 16 + h * 8 + 8],
                            usq[:].rearrange("p (j f) -> p j f", j=8),
                            axis=mybir.AxisListType.X, op=ALU.add)
                        nc.scalar.activation(
                            u_sb[g][:, h * 512:(h + 1) * 512], u_ps[h][:],
                            AF.Copy)

            nc.scalar.sqrt(nrm[:], s1a[:])
            nc.vector.reciprocal(rinv1[:], nrm[:])

            # ================= PASS B =================
            with tc.tile_pool(name="pB", bufs=2, space="PSUM") as pb:
                for g in range(NG):
                    xu = rp.tile([128, 1024], BF16, tag="xu", name="xu")
                    a0 = u_sb[g][:].rearrange("p (j f) -> p j f", j=16)
                    a1 = rinv1[:, g * 16:(g + 1) * 16].rearrange(
                        "p (j o) -> p j o", o=1)
                    b0, b1 = bass.broadcast_tensor_aps(a0, a1)
                    nc.vector.tensor_tensor(
                        xu[:].rearrange("p (j f) -> p j f", j=16), b0, b1,
                        op=ALU.mult)
                    h1 = rp.tile([128, 1024], BF16, tag="h1", name="h1")
                    nc.scalar.activation(h1[:], xu[:], AF.Tanh)
                    h1f = hf[("h1f", g % 2)]
                    ht = [pb.tile([64, 1024], BF16, tag="ht", name="ht")
                          for _ in range(2)]
                    for j in range(16):
                        nc.tensor.matmul(
                            ht[j // 8][0:64, (j % 8) * 128:(j % 8 + 1) * 128],
                            h1[:, j * 64:(j + 1) * 64], id_bf[:],
                            is_transpose=True, skip_group_check=True)
                    for h in range(2):
                        nc.vector.tensor_copy(
                            h1f[0:64, h * 1024:(h + 1) * 1024], ht[h][:])
                    u2 = [pb.tile([128, 512], F32, tag="u2", name="u2")
                          for _ in range(2)]
                    for j in range(16):
                        nc.tensor.matmul(
                            u2[j // 8][:, (j % 8) * 64:(j % 8 + 1) * 64],
                            h1f[:, j * 128:(j + 1) * 128], w_up2[:],
                            skip_group_check=True)
                    h2 = rp.tile([128, 1024], BF16, tag="h2", name="h2")
                    for h in range(2):
                        nc.scalar.activation(
                            h2[:, h * 512:(h + 1) * 512], u2[h][:], AF.Tanh)
                    h2f = hf[("h2f", g % 2)]
                    ht2 = [pb.tile([64, 1024], BF16, tag="ht", name="ht2")
                           for _ in range(2)]
                    for j in range(16):
                        nc.tensor.matmul(
                            ht2[j // 8][0:64, (j % 8) * 128:(j % 8 + 1) * 128],
                            h2[:, j * 64:(j + 1) * 64], id_bf[:],
                            is_transpose=True, skip_group_check=True)
                    nc.vector.tensor_copy(h2f[0:64, 0:1024], ht2[0][:])
                    nc.scalar.activation(h2f[0:64, 1024:2048], ht2[1][:],
                                         AF.Copy)
                    v_ps = pb.tile([128, 512], F32, tag="v", name="v")
                    for j in range(16):
                        nc.tensor.matmul(
                            v_ps[:, j * 32:(j + 1) * 32],
                            h2f[:, j * 128:(j + 1) * 128], w_up3[:],
                            skip_group_check=True)
                    vsq = rp.tile([128, 512], BF16, tag="vsq", name="vsq")
                    nc.scalar.activation(vsq[:], v_ps[:], AF.Square)
                    nc.vector.tensor_reduce(
                        s2a[:, g * 16:(g + 1) * 16],
                        vsq[:].rearrange("p (j f) -> p j f", j=16),
                        axis=mybir.AxisListType.X, op=ALU.add)
                    nc.vector.tensor_copy(v_sb[g][:], v_ps[:])

            nc.scalar.sqrt(nrm[:], s2a[:])
            nc.vector.reciprocal(rinv2[:], nrm[:])

            # ================= PASS C =================
            with (
                tc.tile_pool(name="pCx", bufs=2, space="PSUM") as pcx,
                tc.tile_pool(name="pCz1", bufs=3, space="PSUM") as pz1,
                tc.tile_pool(name="pCz2", bufs=3, space="PSUM") as pz2,
            ):
                x37s = {}

                def chunk_front(c):
                    """msg, x37 assembly, L1 + z1 relus for chunk c."""
                    g, qq = c // 4, c % 4
                    par = c % 2
                    ms = rp.tile([128, 4 * 37], BF16, tag="ms", name="ms")
                    msv = ms[:].rearrange("p (n f) -> p n f", n=4)
                    a0 = v_sb[g][:, qq * 128:(qq + 1) * 128].rearrange(
                        "p (n f) -> p n f", n=4)
                    a1 = rinv2[:, g * 16 + qq * 4:g * 16 + qq * 4 + 4]\
                        .rearrange("p (n o) -> p n o", o=1)
                    b0, b1 = bass.broadcast_tensor_aps(a0, a1)
                    nc.vector.tensor_tensor(msv[:, :, 0:32], b0, b1,
                                            op=ALU.mult)
                    nc.gpsimd.tensor_copy(
                        msv[:, :, 32:35],
                        gsb[g][:].rearrange("p (j f) -> p j f", j=16)
                        [:, qq * 4:qq * 4 + 4, 0:3])
                    nc.gpsimd.tensor_copy(
                        msv[:, :, 35:37],
                        act_sb[:].rearrange("p (c n t) -> p c n t",
                                            c=16, n=4)[:, c, :, :])
                    x37t = pcx.tile([37, 512], BF16, tag="x37t", name="x37t")
                    for n in range(4):
                        nc.tensor.matmul(
                            x37t[:, n * 128:(n + 1) * 128],
                            ms[:, n * 37:(n + 1) * 37], id_bf[:],
                            is_transpose=True, skip_group_check=True)
                    x37 = rp.tile([37, 512], BF16, tag="x37", name="x37")
                    nc.vector.tensor_copy(x37[:], x37t[:])
                    x37s[c] = x37

                    # L1: relu engines 1=Scalar 0=Vector (5 S, 3 V)
                    z1_eng = [1, 0, 1, 1, 0, 1, 1, 0]
                    ri = 0
                    for q in range(2):
                        for mm in range(3):
                            z1p = pz1.tile([128, 512], F32, tag="z1p",
                                           name="z1p")
                            nc.tensor.matmul(
                                z1p[:], w_q1[q][:, mm * 128:(mm + 1) * 128],
                                x37[:])
                            if z1_eng[ri]:
                                nc.scalar.activation(
                                    z1sb[(q, mm, par)][:], z1p[:], AF.Relu)
                            else:
                                nc.vector.tensor_scalar_max(
                                    z1sb[(q, mm, par)][:], z1p[:], 0.0)
                            ri += 1
                    z1px = pz1.tile([128, 512], F32, tag="z1p", name="z1px")
                    nc.tensor.matmul(z1px[0:48, :], w_q1x[:], x37[:])
                    nc.scalar.activation(z1sb[(0, 3, par)][0:16, :],
                                         z1px[0:16, :], AF.Relu)
                    nc.vector.tensor_scalar_max(z1sb[(1, 3, par)][0:16, :],
                                                z1px[32:48, :], 0.0)

                def chunk_l2(c):
                    """L2 + fused q3 for chunk c."""
                    par = c % 2
                    for q in range(2):
                        for bt in range(4):
                            z2p = pz2.tile([128, 300], F32, tag="z2p",
                                           name="z2p")
                            for k in range(4):
                                nc.tensor.matmul(
                                    z2p[:],
                                    z1sb[(q, k, par)]
                                    [:, bt * 128:(bt + 1) * 128],
                                    w_q2[q][k][:],
                                    start=(k == 0), stop=(k == 3))
                            col = c * 8 + q * 4 + bt
                            z2s = rp.tile([128, 300], BF16, tag="z2s",
                                          name="z2s")
                            nc.vector.scalar_tensor_tensor(
                                out=z2s[:], in0=z2p[:], scalar=0.0,
                                in1=w_3[q][:], op0=ALU.max, op1=ALU.mult,
                                accum_out=xq[:, col:col + 1])

                chunk_front(0)
                for c in range(1, NCH):
                    chunk_front(c)
                    chunk_l2(c - 1)
                    del x37s[c - 1]
                chunk_l2(NCH - 1)

            nc.sync.dma_start(out[:], xq[:])

    nc.compile()
    _prog_cache["nc"] = nc
    return nc


def _prep_weights(inputs):
    bf = ml_dtypes.bfloat16
    f32 = np.float32

    def cat(*xs):
        return np.ascontiguousarray(
            np.concatenate([np.asarray(x, f32) for x in xs], 0)).astype(bf)

    w = {}
    w["wup1"] = cat(inputs["up_fc1_w"], inputs["up_fc1_b"][None])
    w["wup2"] = cat(inputs["up_fc2_w"][:64], inputs["up_fc2_b"][None])
    w["wup3"] = cat(inputs["up_fc3_w"], inputs["up_fc3_b"][None])
    q1x = []
    for qi, q in enumerate(("q1", "q2")):
        tag = "a" if qi == 0 else "b"
        w1 = np.asarray(inputs[q + "_w1"], f32)
        # x37 rows: msg(32), pos(3), act(1), one(1)
        full = np.concatenate(
            [w1[12:44], w1[6:9] + w1[9:12], w1[44:45],
             np.asarray(inputs[q + "_b1"], f32)[None]], 0)  # [37, 400]
        w[f"w{tag}1"] = np.ascontiguousarray(full[:, 0:384]).astype(bf)
        q1x.append(full[:, 384:400] if qi else np.concatenate(
            [full[:, 384:400], np.zeros((37, 16), f32)], 1))
        w[f"w{tag}2"] = cat(inputs[q + "_w2"], np.zeros((16, 300), f32),
                            inputs[q + "_b2"][None])
        w3 = np.asarray(inputs[q + "_w3"], f32)[:, 0]
        w[f"w{tag}3"] = np.ascontiguousarray(
            np.tile(w3[None, :], (128, 1))).astype(bf)
    w["wq1x"] = np.ascontiguousarray(np.concatenate(q1x, 1)).astype(bf)
    w["idbf"] = np.eye(128, dtype=f32).astype(bf)
    return w


def _core_inputs(w, state, action, core):
    m = dict(w)
    m["state"] = np.ascontiguousarray(state[core * BC:(core + 1) * BC])
    a = action[core * BC:(core + 1) * BC]
    # actp[p, (c, n, {act,one})]; row = g*2048 + p*16 + qq*4 + n
    ar = a.reshape(4, 128, 4, 4)            # [g, p, qq, n]
    ap_ = ar.transpose(1, 0, 2, 3).reshape(128, 16, 4)
    acts = np.stack([ap_, np.ones_like(ap_)], -1)
    m["actp"] = np.ascontiguousarray(
        acts.reshape(128, 128)).astype(ml_dtypes.bfloat16)
    return m


def _run(inputs, trace=False):
    nc = build_program()
    w = _prep_weights(inputs)
    state = np.ascontiguousarray(np.asarray(inputs["state"], np.float32))
    action = np.asarray(inputs["action"], np.float32).reshape(-1)
    in_maps = [_core_inputs(w, state, action, core) for core in range(NCORES)]
    res = run_bass_kernel_spmd(nc, in_maps, list(range(NCORES)), trace=trace)
    b1 = float(np.asarray(inputs["q1_b3"])[0])
    b2 = float(np.asarray(inputs["q2_b3"])[0])
    x1 = np.empty((NCORES, BC), np.float32)
    x2 = np.empty((NCORES, BC), np.float32)
    for core in range(NCORES):
        o = res.results[core]["out"]            # [128, 128]
        t = o.reshape(128, 4, 4, 2, 4)          # [p, g, qq, q, bt(=n)]
        xx = t.transpose(3, 1, 0, 2, 4).reshape(2, BC)  # row g*2048+p*16+qq*4+n
        x1[core] = xx[0] + b1
        x2[core] = xx[1] + b2
    return (x1.reshape(-1, 1), x2.reshape(-1, 1)), res


def kernel(**inputs):
    (x1, x2), _ = _run(inputs)
    return x1, x2


# revision 3
# speedup vs baseline: 1.2213x; 1.2213x over previous
"""Trainium2 Bass kernel for nn_CriticGraphPolicy (single-limb critic GNN).

Live graph per row (state s[32], action a[1]):
  u  = [s;a;1] @ Wup1;  xu = u/||u||;  h1 = tanh(xu)
  h2 = tanh([h1;1] @ Wup2);  v = [h2;1] @ Wup3;  m = v/||v||
  x37 = [m; s[:3]; a; 1]
  z1 = relu(x37 @ Wq1);  z2 = relu([z1;1] @ Wq2);  xq = z2@w3 + b3
for q in {q1, q2}.

Batch layout: row r of a core's 8192 = g*2048 + p*16 + j  (g group,
p partition, j = qq*4 + n; chunk c = g*4 + qq covers 512 rows as
columns n*128 + p).  Host un-permutes at the end and adds b3.

Structure: 3 passes so the Scalar engine never ping-pongs activation
tables (sqrt is batched between passes; tanh/square/relu/copy share
tables within each pass):
  pass A (per group g): load+transpose input, up1, |u|^2 row-sums
  sqrt+recip (all 64 rowgroups at once)
  pass B: xu=u*rinv, tanh, up2, tanh, up3, |v|^2 sums
  sqrt+recip
  pass C (per 512-chunk, software-pipelined one chunk deep so the PE
  never waits on relu results): msg=v*rinv2, build x37, L1(fm), relu,
  then L2(bm) of the PREVIOUS chunk + fused q3 (relu*w3+rowsum in one
  scalar_tensor_tensor on DVE; two of eight via Scalar relu + GpSimd
  mul/reduce to balance engines).
"""

import numpy as np
import ml_dtypes

import concourse.bass as bass
import concourse.bacc as bacc
import concourse.mybir as mybir
from concourse import tile
from concourse.bass_utils import run_bass_kernel_spmd

F32 = mybir.dt.float32
BF16 = mybir.dt.bfloat16

NCORES = 8
B = 65536
BC = B // NCORES          # 8192 rows per core
NG = 4                    # groups of 2048 rows (super-chunks for A/B)
GR = BC // NG             # 2048
NCH = 16                  # pass-C chunks of 512

_prog_cache = {}


def build_program():
    if "nc" in _prog_cache:
        return _prog_cache["nc"]
    nc = bacc.Bacc("TRN2", target_bir_lowering=False, debug=False,
                   num_devices=NCORES)

    state = nc.dram_tensor("state", [BC, 32], F32, kind="ExternalInput").ap()
    actp = nc.dram_tensor("actp", [128, 128], BF16, kind="ExternalInput").ap()
    wup1d = nc.dram_tensor("wup1", [34, 64], BF16, kind="ExternalInput").ap()
    wup2d = nc.dram_tensor("wup2", [65, 64], BF16, kind="ExternalInput").ap()
    wup3d = nc.dram_tensor("wup3", [65, 32], BF16, kind="ExternalInput").ap()
    # q-head weights: L1 main tiles [37, 384] per q + merged extra [37, 48]
    wq1d = [nc.dram_tensor(f"w{q}1", [37, 384], BF16, kind="ExternalInput").ap()
            for q in ("a", "b")]
    wq1xd = nc.dram_tensor("wq1x", [37, 48], BF16, kind="ExternalInput").ap()
    # L2: [417, 300] per q (400 z1 + 16 pad + bias row at 416)
    wq2d = [nc.dram_tensor(f"w{q}2", [417, 300], BF16, kind="ExternalInput").ap()
            for q in ("a", "b")]
    w3d = [nc.dram_tensor(f"w{q}3", [128, 300], BF16, kind="ExternalInput").ap()
           for q in ("a", "b")]
    idbf = nc.dram_tensor("idbf", [128, 128], BF16, kind="ExternalInput").ap()
    zbias = nc.dram_tensor("zbias", [33, 512], BF16, kind="ExternalInput").ap()
    onesr = nc.dram_tensor("onesr", [1, 2048], BF16, kind="ExternalInput").ap()
    out = nc.dram_tensor("out", [128, 128], F32, kind="ExternalOutput").ap()

    AF = mybir.ActivationFunctionType
    ALU = mybir.AluOpType
    KS = [128, 128, 128, 33]          # L2 k-tiles (400 z1 + 16 pad + bias@32)

    with tile.TileContext(nc) as tc:
        with (
            tc.tile_pool(name="wp", bufs=1) as wp,
            tc.tile_pool(name="rot", bufs=2) as rp,
        ):
            # ---- static weights / persistent state ----
            def ldw(tag, src, shape):
                t = wp.tile(shape, BF16, tag=tag, name=tag)
                nc.sync.dma_start(t[:], src)
                return t
            # inputs + pass-A weights first: pass A can start as soon as
            # gsb[0]/actp/idbf/wup1 land; q-head weights are not needed
            # until pass C (~100us later).
            def gsb_load(g):
                t = wp.tile([128, GR * 32 // 128], F32, tag=f"gsb{g}",
                            name=f"gsb{g}")
                nc.sync.dma_start(
                    t[:], state[g * GR:(g + 1) * GR, :].rearrange(
                        "(p j) f -> p (j f)", p=128))
                return t
            gsb = [gsb_load(g) for g in range(NG)]
            act_sb = ldw("actp", actp[:], [128, 128])
            id_bf = ldw("idbf", idbf[:], [128, 128])
            w_up1 = ldw("wup1", wup1d[:], [34, 64])
            w_up2 = ldw("wup2", wup2d[:], [65, 64])
            w_up3 = ldw("wup3", wup3d[:], [65, 32])
            w_q1 = [ldw(f"wq1_{q}", wq1d[q][:], [37, 384]) for q in range(2)]
            w_q1x = ldw("wq1x", wq1xd[:], [37, 48])
            w_q2 = []
            for q in range(2):
                tiles = []
                off = 0
                for k in range(4):
                    t = ldw(f"wq2_{q}_{k}", wq2d[q][off:off + KS[k], :],
                            [KS[k], 300])
                    tiles.append(t)
                    off += KS[k]
                w_q2.append(tiles)
            w_3 = [ldw(f"w3_{q}", w3d[q][:], [128, 300]) for q in range(2)]

            u_sb = [wp.tile([128, 1024], BF16, tag=f"usb{g}", name=f"usb{g}")
                    for g in range(NG)]
            v_sb = [wp.tile([128, 512], BF16, tag=f"vsb{g}", name=f"vsb{g}")
                    for g in range(NG)]
            s1a = wp.tile([128, 64], F32, tag="s1a", name="s1a")
            s2a = wp.tile([128, 64], F32, tag="s2a", name="s2a")
            nrm = wp.tile([128, 64], F32, tag="nrm", name="nrm")
            rinv1 = wp.tile([128, 64], F32, tag="rinv1", name="rinv1")
            rinv2 = wp.tile([128, 64], F32, tag="rinv2", name="rinv2")
            xq = wp.tile([128, 128], F32, tag="xq", name="xq")

            # h1f/h2f feature-major activations (+ones row), parity-buffered
            hf = {}
            for nm in ("h1f", "h2f"):
                for par in range(2):
                    t = wp.tile([65, 2048], BF16, tag=f"{nm}_{par}",
                                name=f"{nm}_{par}")
                    nc.sync.dma_start(t[64:65, :], onesr[:])
                    hf[(nm, par)] = t
            # z1 tiles (feature-major relu outputs), parity-buffered
            z1sb = {}
            for q in range(2):
                for par in range(2):
                    for k in range(4):
                        t = wp.tile([KS[k], 512], BF16, tag=f"z1_{q}{k}{par}",
                                    name=f"z1_{q}{k}{par}")
                        if k == 3:
                            # rows 0:32 zero pad, row 32 = L2 bias ones
                            nc.sync.dma_start(t[:], zbias[:])
                        z1sb[(q, k, par)] = t

            # ================= PASS A =================
            with tc.tile_pool(name="pA", bufs=2, space="PSUM") as pa:
                for g in range(NG):
                    xl = rp.tile([128, 16 * 34], BF16, tag="xl", name="xl")
                    nc.gpsimd.tensor_copy(
                        xl[:].rearrange("p (j f) -> p j f", j=16)[:, :, 0:32],
                        gsb[g][:].rearrange("p (j f) -> p j f", j=16))
                    nc.gpsimd.tensor_copy(
                        xl[:].rearrange("p (j f) -> p j f", j=16)[:, :, 32:34],
                        act_sb[:].rearrange("p (c j t) -> p c j t", c=16, j=4)
                        [:, g * 4:(g + 1) * 4, :, :].rearrange(
                            "p c j t -> p (c j) t"))
                    x34t = [pa.tile([34, 1024], BF16, tag="x34t", name="x34t")
                            for _ in range(2)]
                    for j in range(16):
                        nc.tensor.matmul(
                            x34t[j // 8][:, (j % 8) * 128:(j % 8 + 1) * 128],
                            xl[:, j * 34:(j + 1) * 34], id_bf[:],
                            is_transpose=True, skip_group_check=True)
                    x34 = rp.tile([34, 2048], BF16, tag="x34", name="x34")
                    for h in range(2):
                        nc.vector.tensor_copy(
                            x34[:, h * 1024:(h + 1) * 1024], x34t[h][:])
                    u_ps = [pa.tile([128, 512], F32, tag="u", name="u")
                            for _ in range(2)]
                    for j in range(16):
                        nc.tensor.matmul(
                            u_ps[j // 8][:, (j % 8) * 64:(j % 8 + 1) * 64],
                            x34[:, j * 128:(j + 1) * 128], w_up1[:],
                            skip_group_check=True)
                    for h in range(2):
                        usq = rp.tile([128, 512], BF16, tag="usq", name="usq")
                        nc.scalar.activation(usq[:], u_ps[h][:], AF.Square)
                        nc.vector.tensor_reduce(
                            s1a[:, g * 16 + h * 8:g * 16 + h * 8 + 8],
                            usq[:].rearrange("p (j f) -> p j f", j=8),
                            axis=mybir.AxisListType.X, op=ALU.add)
                        nc.scalar.activation(
                            u_sb[g][:, h * 512:(h + 1) * 512], u_ps[h][:],
                            AF.Copy)

            nc.scalar.sqrt(nrm[:], s1a[:])
            nc.vector.reciprocal(rinv1[:], nrm[:])

            # ================= PASS B =================
            with tc.tile_pool(name="pB", bufs=2, space="PSUM") as pb:
                for g in range(NG):
                    xu = rp.tile([128, 1024], BF16, tag="xu", name="xu")
                    a0 = u_sb[g][:].rearrange("p (j f) -> p j f", j=16)
                    a1 = rinv1[:, g * 16:(g + 1) * 16].rearrange(
                        "p (j o) -> p j o", o=1)
                    b0, b1 = bass.broadcast_tensor_aps(a0, a1)
                    nc.vector.tensor_tensor(
                        xu[:].rearrange("p (j f) -> p j f", j=16), b0, b1,
                        op=ALU.mult)
                    h1 = rp.tile([128, 1024], BF16, tag="h1", name="h1")
                    nc.scalar.activation(h1[:], xu[:], AF.Tanh)
                    h1f = hf[("h1f", g % 2)]
                    ht = [pb.tile([64, 1024], BF16, tag="ht", name="ht")
                          for _ in range(2)]
                    for j in range(16):
                        nc.tensor.matmul(
                            ht[j // 8][0:64, (j % 8) * 128:(j % 8 + 1) * 128],
                            h1[:, j * 64:(j + 1) * 64], id_bf[:],
                            is_transpose=True, skip_group_check=True)
                    for h in range(2):
                        nc.vector.tensor_copy(
                            h1f[0:64, h * 1024:(h + 1) * 1024], ht[h][:])
                    u2 = [pb.tile([128, 512], F32, tag="u2", name="u2")
                          for _ in range(2)]
                    for j in range(16):
                        nc.tensor.matmul(
                            u2[j // 8][:, (j % 8) * 64:(j % 8 + 1) * 64],
                            h1f[:, j * 128:(j + 1) * 128], w_up2[:],
                            skip_group_check=True)
                    h2 = rp.tile([128, 1024], BF16, tag="h2", name="h2")
                    for h in range(2):
                        nc.scalar.activation(
                            h2[:, h * 512:(h + 1) * 512], u2[h][:], AF.Tanh)
                    h2f = hf[("h2f", g % 2)]
                    ht2 = [pb.tile([64, 1024], BF16, tag="ht", name="ht2")
                           for _ in range(2)]
                    for j in range(16):
                        nc.tensor.matmul(
                            ht2[j // 8][0:64, (j % 8) * 128:(j % 8 + 1) * 128],
                            h2[:, j * 64:(j + 1) * 64], id_bf[:],
                            is_transpose=True, skip_group_check=True)
                    nc.vector.tensor_copy(h2f[0:64, 0:1024], ht2[0][:])
                    nc.scalar.activation(h2f[0:64, 1024:2048], ht2[1][:],
                                         AF.Copy)
                    v_ps = pb.tile([128, 512], F32, tag="v", name="v")
                    for j in range(16):
                        nc.tensor.matmul(
                            v_ps[:, j * 32:(j + 1) * 32],
                            h2f[:, j * 128:(j + 1) * 128], w_up3[:],
                            skip_group_check=True)
                    vsq = rp.tile([128, 512], BF16, tag="vsq", name="vsq")
                    nc.scalar.activation(vsq[:], v_ps[:], AF.Square)
                    nc.vector.tensor_reduce(
                        s2a[:, g * 16:(g + 1) * 16],
                        vsq[:].rearrange("p (j f) -> p j f", j=16),
                        axis=mybir.AxisListType.X, op=ALU.add)
                    nc.vector.tensor_copy(v_sb[g][:], v_ps[:])

            nc.scalar.sqrt(nrm[:], s2a[:])
            nc.vector.reciprocal(rinv2[:], nrm[:])

            # ================= PASS C =================
            with (
                tc.tile_pool(name="pCx", bufs=2, space="PSUM") as pcx,
                tc.tile_pool(name="pCz1", bufs=3, space="PSUM") as pz1,
                tc.tile_pool(name="pCz2", bufs=3, space="PSUM") as pz2,
            ):
                x37s = {}

                def chunk_front(c):
                    """msg, x37 assembly, L1 + z1 relus for chunk c."""
                    g, qq = c // 4, c % 4
                    par = c % 2
                    ms = rp.tile([128, 4 * 37], BF16, tag="ms", name="ms")
                    msv = ms[:].rearrange("p (n f) -> p n f", n=4)
                    a0 = v_sb[g][:, qq * 128:(qq + 1) * 128].rearrange(
                        "p (n f) -> p n f", n=4)
                    a1 = rinv2[:, g * 16 + qq * 4:g * 16 + qq * 4 + 4]\
                        .rearrange("p (n o) -> p n o", o=1)
                    b0, b1 = bass.broadcast_tensor_aps(a0, a1)
                    nc.vector.tensor_tensor(msv[:, :, 0:32], b0, b1,
                                            op=ALU.mult)
                    nc.gpsimd.tensor_copy(
                        msv[:, :, 32:35],
                        gsb[g][:].rearrange("p (j f) -> p j f", j=16)
                        [:, qq * 4:qq * 4 + 4, 0:3])
                    nc.gpsimd.tensor_copy(
                        msv[:, :, 35:37],
                        act_sb[:].rearrange("p (c n t) -> p c n t",
                                            c=16, n=4)[:, c, :, :])
                    x37t = pcx.tile([37, 512], BF16, tag="x37t", name="x37t")
                    for n in range(4):
                        nc.tensor.matmul(
                            x37t[:, n * 128:(n + 1) * 128],
                            ms[:, n * 37:(n + 1) * 37], id_bf[:],
                            is_transpose=True, skip_group_check=True)
                    x37 = rp.tile([37, 512], BF16, tag="x37", name="x37")
                    nc.vector.tensor_copy(x37[:], x37t[:])
                    x37s[c] = x37

                    # L1: relu engines 1=Scalar 0=Vector (5 S, 3 V)
                    z1_eng = [1, 0, 1, 1, 0, 1, 1, 0]
                    ri = 0
                    for q in range(2):
                        for mm in range(3):
                            z1p = pz1.tile([128, 512], F32, tag="z1p",
                                           name="z1p")
                            nc.tensor.matmul(
                                z1p[:], w_q1[q][:, mm * 128:(mm + 1) * 128],
                                x37[:])
                            if z1_eng[ri]:
                                nc.scalar.activation(
                                    z1sb[(q, mm, par)][:], z1p[:], AF.Relu)
                            else:
                                nc.vector.tensor_scalar_max(
                                    z1sb[(q, mm, par)][:], z1p[:], 0.0)
                            ri += 1
                    z1px = pz1.tile([128, 512], F32, tag="z1p", name="z1px")
                    nc.tensor.matmul(z1px[0:48, :], w_q1x[:], x37[:])
                    nc.scalar.activation(z1sb[(0, 3, par)][0:16, :],
                                         z1px[0:16, :], AF.Relu)
                    nc.vector.tensor_scalar_max(z1sb[(1, 3, par)][0:16, :],
                                                z1px[32:48, :], 0.0)

                def chunk_l2(c):
                    """L2 + fused q3 for chunk c."""
                    par = c % 2
                    for q in range(2):
                        for bt in range(4):
                            z2p = pz2.tile([128, 300], F32, tag="z2p",
                                           name="z2p")
                            for k in range(4):
                                nc.tensor.matmul(
                                    z2p[:],
                                    z1sb[(q, k, par)]
                                    [:, bt * 128:(bt + 1) * 128],
                                    w_q2[q][k][:],
                                    start=(k == 0), stop=(k == 3))
                            col = c * 8 + q * 4 + bt
                            z2s = rp.tile([128, 300], BF16, tag="z2s",
                                          name="z2s")
                            nc.vector.scalar_tensor_tensor(
                                out=z2s[:], in0=z2p[:], scalar=0.0,
                                in1=w_3[q][:], op0=ALU.max, op1=ALU.mult,
                                accum_out=xq[:, col:col + 1])

                chunk_front(0)
                for c in range(1, NCH):
                    chunk_front(c)
                    chunk_l2(c - 1)
                    del x37s[c - 1]
                chunk_l2(NCH - 1)

            nc.sync.dma_start(out[:], xq[:])

    nc.compile()
    _prog_cache["nc"] = nc
    return nc


def _prep_weights(inputs):
    bf = ml_dtypes.bfloat16
    f32 = np.float32

    def cat(*xs):
        return np.ascontiguousarray(
            np.concatenate([np.asarray(x, f32) for x in xs], 0)).astype(bf)

    w = {}
    w["wup1"] = cat(inputs["up_fc1_w"], inputs["up_fc1_b"][None])
    w["wup2"] = cat(inputs["up_fc2_w"][:64], inputs["up_fc2_b"][None])
    w["wup3"] = cat(inputs["up_fc3_w"], inputs["up_fc3_b"][None])
    q1x = []
    for qi, q in enumerate(("q1", "q2")):
        tag = "a" if qi == 0 else "b"
        w1 = np.asarray(inputs[q + "_w1"], f32)
        # x37 rows: msg(32), pos(3), act(1), one(1)
        full = np.concatenate(
            [w1[12:44], w1[6:9] + w1[9:12], w1[44:45],
             np.asarray(inputs[q + "_b1"], f32)[None]], 0)  # [37, 400]
        w[f"w{tag}1"] = np.ascontiguousarray(full[:, 0:384]).astype(bf)
        q1x.append(full[:, 384:400] if qi else np.concatenate(
            [full[:, 384:400], np.zeros((37, 16), f32)], 1))
        w[f"w{tag}2"] = cat(inputs[q + "_w2"], np.zeros((16, 300), f32),
                            inputs[q + "_b2"][None])
        w3 = np.asarray(inputs[q + "_w3"], f32)[:, 0]
        w[f"w{tag}3"] = np.ascontiguousarray(
            np.tile(w3[None, :], (128, 1))).astype(bf)
    w["wq1x"] = np.ascontiguousarray(np.concatenate(q1x, 1)).astype(bf)
    w["idbf"] = np.eye(128, dtype=f32).astype(bf)
    zb = np.zeros((33, 512), f32)
    zb[32, :] = 1.0
    w["zbias"] = zb.astype(bf)
    w["onesr"] = np.ones((1, 2048), f32).astype(bf)
    return w


def _core_inputs(w, state, action, core):
    m = dict(w)
    m["state"] = np.ascontiguousarray(state[core * BC:(core + 1) * BC])
    a = action[core * BC:(core + 1) * BC]
    # actp[p, (c, n, {act,one})]; row = g*2048 + p*16 + qq*4 + n
    ar = a.reshape(4, 128, 4, 4)            # [g, p, qq, n]
    ap_ = ar.transpose(1, 0, 2, 3).reshape(128, 16, 4)
    acts = np.stack([ap_, np.ones_like(ap_)], -1)
    m["actp"] = np.ascontiguousarray(
        acts.reshape(128, 128)).astype(ml_dtypes.bfloat16)
    return m


def _run(inputs, trace=False):
    nc = build_program()
    w = _prep_weights(inputs)
    state = np.ascontiguousarray(np.asarray(inputs["state"], np.float32))
    action = np.asarray(inputs["action"], np.float32).reshape(-1)
    in_maps = [_core_inputs(w, state, action, core) for core in range(NCORES)]
    res = run_bass_kernel_spmd(nc, in_maps, list(range(NCORES)), trace=trace)
    b1 = float(np.asarray(inputs["q1_b3"])[0])
    b2 = float(np.asarray(inputs["q2_b3"])[0])
    x1 = np.empty((NCORES, BC), np.float32)
    x2 = np.empty((NCORES, BC), np.float32)
    for core in range(NCORES):
        o = res.results[core]["out"]            # [128, 128]
        t = o.reshape(128, 4, 4, 2, 4)          # [p, g, qq, q, bt(=n)]
        xx = t.transpose(3, 1, 0, 2, 4).reshape(2, BC)  # row g*2048+p*16+qq*4+n
        x1[core] = xx[0] + b1
        x2[core] = xx[1] + b2
    return (x1.reshape(-1, 1), x2.reshape(-1, 1)), res


def kernel(**inputs):
    (x1, x2), _ = _run(inputs)
    return x1, x2
